# revision 7
# baseline (speedup 1.0000x reference)
"""GQA Trainium2 Bass kernel.

Sharding: 8 cores = 2 batches x 4 KV groups. Each core computes, for its
(b, g): qT = Wq_g^T X_q^T (4 heads, [128, S] each), kT, v; then per head
scores^T = kT_tile^T qT (s2-major), exp via ACT, softmax denominators via
ones-matmul on PE, AV accumulation (out attnT [hd, S]), normalization on
PSUM eviction, and the Wo row-shard partial product [S, E]. Host sums the
4 group partials per batch and adds bo.

All matmuls run in float32r (TF32-like, 1 cycle/row at N=512, ~1.5e-4 rel
err). Inputs are fed as X^T (transposed on host) so every DMA is a clean
128-partition strided load.
"""
import sys
sys.path.insert(0, '/opt/trn_rl_repo')
from contextlib import ExitStack

import numpy as np

import concourse.bass as bass
import concourse.tile as tile
from concourse import bacc, mybir
from concourse.masks import make_identity

E, NH, G, HD = 2048, 16, 4, 128
KV = E // G            # 512
B, S = 2, 2048
MQ = (NH // G) * HD    # 512 q columns per group
P = 128
SC = S // 512          # 4 s-chunks of 512
ECN = E // P           # 16 contraction chunks
NT = S // P            # 16 s2 tiles
H = NH // G            # 4 heads per core
N_CORES = 8
F32 = mybir.dt.float32
F32R = mybir.dt.float32r
SCALE = float(HD) ** -0.5
EXPF = mybir.ActivationFunctionType.Exp
COPYF = mybir.ActivationFunctionType.Copy


def build(loop_trips=None):
    """Build the per-core Bass program. loop_trips wraps the whole body in a
    For_i for wall-clock timing (amortizes the axon proxy overhead)."""
    nc = bacc.Bacc("TRN2", target_bir_lowering=False, debug=False,
                   num_devices=N_CORES)

    xq_d = nc.dram_tensor("xq", [E, S], F32R, kind="ExternalInput").ap()
    xk_d = nc.dram_tensor("xk", [E, S], F32R, kind="ExternalInput").ap()
    xv_d = nc.dram_tensor("xv", [E, S], F32R, kind="ExternalInput").ap()
    wq_d = nc.dram_tensor("wq", [E, MQ], F32R, kind="ExternalInput").ap()
    wk_d = nc.dram_tensor("wk", [E, HD], F32R, kind="ExternalInput").ap()
    wv_d = nc.dram_tensor("wv", [E, HD], F32R, kind="ExternalInput").ap()
    wo_d = nc.dram_tensor("wo", [MQ, E], F32R, kind="ExternalInput").ap()
    bq_d = nc.dram_tensor("bq", [MQ, 1], F32, kind="ExternalInput").ap()
    bk_d = nc.dram_tensor("bk", [HD, 1], F32, kind="ExternalInput").ap()
    bv_d = nc.dram_tensor("bv", [HD, 1], F32, kind="ExternalInput").ap()
    out_d = nc.dram_tensor("out", [S, E], F32, kind="ExternalOutput").ap()

    with tile.TileContext(nc) as tc:
        with ExitStack() as ctx:
            # pools
            big = ctx.enter_context(tc.tile_pool(name="big", bufs=22))
            wsm = ctx.enter_context(tc.tile_pool(name="wsm", bufs=16))
            wop = ctx.enter_context(tc.tile_pool(name="wop", bufs=16))
            qtp = ctx.enter_context(tc.tile_pool(name="qtp", bufs=1))
            ktp = ctx.enter_context(tc.tile_pool(name="ktp", bufs=1))
            vp = ctx.enter_context(tc.tile_pool(name="vp", bufs=16))
            atp = ctx.enter_context(tc.tile_pool(name="atp", bufs=1))
            xp = ctx.enter_context(tc.tile_pool(name="xp", bufs=4))
            vsbp = ctx.enter_context(tc.tile_pool(name="vsbp", bufs=2))
            outp = ctx.enter_context(tc.tile_pool(name="outp", bufs=4))
            smp = ctx.enter_context(tc.tile_pool(name="smp", bufs=1))
            ps = ctx.enter_context(tc.tile_pool(name="ps", bufs=8, space="PSUM"))

            # constants / weights (loop invariant)
            ident_f = smp.tile([P, P], F32, tag="ident_f")
            make_identity(nc, ident_f[:])
            ident = smp.tile([P, P], F32R, tag="ident")
            nc.vector.tensor_copy(ident[:], ident_f[:])
            ones_f = smp.tile([P, 1], F32, tag="ones_f")
            nc.vector.memset(ones_f[:], 1.0)
            ones_t = smp.tile([P, 1], F32R, tag="ones")
            nc.vector.tensor_copy(ones_t[:], ones_f[:])
            ones_cf = smp.tile([1, P], F32, tag="ones_cf")
            nc.vector.memset(ones_cf[:], 1.0)
            ones_col = smp.tile([1, P], F32R, tag="ones_col")
            nc.vector.tensor_copy(ones_col[:], ones_cf[:])
            bq_t = []
            for m in range(H):
                bt = smp.tile([P, 1], F32, tag=f"bq{m}")
                nc.sync.dma_start(bt[:], bq_d[m * P:(m + 1) * P, :])
                bq_t.append(bt)
            bk_t = smp.tile([P, 1], F32, tag="bk")
            nc.sync.dma_start(bk_t[:], bk_d[:, :])
            bv_t = smp.tile([P, 1], F32, tag="bv")
            nc.sync.dma_start(bv_t[:], bv_d[:, :])

            wk_t, wv_t, wq_t = [], [], []
            for e in range(ECN):
                t = wsm.tile([P, HD], F32R, tag="wk")
                nc.sync.dma_start(t[:], wk_d[e * P:(e + 1) * P, :])
                wk_t.append(t)
            for e in range(ECN):
                t = wsm.tile([P, HD], F32R, tag="wv")
                nc.sync.dma_start(t[:], wv_d[e * P:(e + 1) * P, :])
                wv_t.append(t)
            for e in range(ECN):
                t = big.tile([P, MQ], F32R, tag="big")
                nc.sync.dma_start(t[:], wq_d[e * P:(e + 1) * P, :])
                wq_t.append(t)
            wo_t = []
            for h in range(H):
                row = []
                for ec in range(4):
                    t = wop.tile([P, 512], F32R, tag="wo")
                    nc.sync.dma_start(
                        t[:], wo_d[h * P:(h + 1) * P, ec * 512:(ec + 1) * 512])
                    row.append(t)
                wo_t.append(row)

            def body(_iv=None):
                # ---- K projection -> kT [128, S] ----
                kT = ktp.tile([P, S], F32R, tag="kT")
                for c in range(SC):
                    ps_k = ps.tile([P, 512], F32, tag="ps")
                    for e in range(ECN):
                        xt = xp.tile([P, 512], F32R, tag="x")
                        nc.sync.dma_start(
                            xt[:], xk_d[e * P:(e + 1) * P, c * 512:(c + 1) * 512])
                        nc.tensor.matmul(ps_k[:], wk_t[e][:], xt[:],
                                         start=(e == 0), stop=(e == ECN - 1))
                    nc.vector.tensor_add(
                        kT[:, c * 512:(c + 1) * 512], ps_k[:],
                        bk_t[:].broadcast_to([P, 512]))

                # ---- V projection -> v tiles [s2, hd] (natural) ----
                v_tiles = []
                for c in range(SC):
                    ps_v = ps.tile([P, 512], F32, tag="ps")
                    for e in range(ECN):
                        xt = xp.tile([P, 512], F32R, tag="x")
                        nc.sync.dma_start(
                            xt[:], xv_d[e * P:(e + 1) * P, c * 512:(c + 1) * 512])
                        nc.tensor.matmul(ps_v[:], wv_t[e][:], xt[:],
                                         start=(e == 0), stop=(e == ECN - 1))
                    vsb = vsbp.tile([P, 512], F32R, tag="vsb")
                    nc.vector.tensor_add(vsb[:], ps_v[:],
                                         bv_t[:].broadcast_to([P, 512]))
                    for t in range(4):
                        pst = ps.tile([P, P], F32R, tag="ps")
                        nc.tensor.transpose(pst[:], vsb[:, t * P:(t + 1) * P],
                                            ident[:])
                        vt = vp.tile([P, P], F32R, tag="v")
                        nc.vector.tensor_copy(vt[:], pst[:])
                        v_tiles.append(vt)

                # ---- Q projection -> qT[h] [128, S] ----
                qT = [qtp.tile([P, S], F32R, tag=f"qT{h}", name=f"qT{h}") for h in range(H)]
                for c in range(SC):
                    ps_q = [ps.tile([P, 512], F32, tag="ps", name=f"psq{c}") for _ in range(H)]
                    for e in range(ECN):
                        xt = xp.tile([P, 512], F32R, tag="x")
                        nc.sync.dma_start(
                            xt[:], xq_d[e * P:(e + 1) * P, c * 512:(c + 1) * 512])
                        for m in range(H):
                            nc.tensor.matmul(
                                ps_q[m][:], wq_t[e][:, m * P:(m + 1) * P], xt[:],
                                start=(e == 0), stop=(e == ECN - 1))
                    for m in range(H):
                        nc.vector.tensor_add(
                            qT[m][:, c * 512:(c + 1) * 512], ps_q[m][:],
                            bq_t[m][:].broadcast_to([P, 512]))

                # ---- attention, pipelined by one (c, h) step ----
                attnT = [atp.tile([P, S], F32R, tag=f"attnT{h}", name=f"attnT{h}")
                         for h in range(H)]

                def emit_score(step, t):
                    c, h = divmod(step, H)
                    pss = ps.tile([P, 512], F32, tag="ps")
                    nc.tensor.matmul(pss[:], kT[:, t * P:(t + 1) * P],
                                     qT[h][:, c * 512:(c + 1) * 512],
                                     start=True, stop=True)
                    ew = big.tile([P, 512], F32R, tag="big")
                    nc.scalar.activation(ew[:], pss[:], EXPF, scale=SCALE)
                    return ew

                def emit_wo(c):
                    for st in range(4):
                        s1t = c * 4 + st
                        for ecx in range(4):
                            pso = ps.tile([P, 512], F32, tag="ps")
                            for hh in range(H):
                                nc.tensor.matmul(
                                    pso[:],
                                    attnT[hh][:, s1t * P:(s1t + 1) * P],
                                    wo_t[hh][ecx][:],
                                    start=(hh == 0), stop=(hh == H - 1))
                            ob = outp.tile([P, 512], F32, tag="ob")
                            nc.scalar.activation(ob[:], pso[:], COPYF)
                            nc.sync.dma_start(
                                out_d[s1t * P:(s1t + 1) * P,
                                      ecx * 512:(ecx + 1) * 512], ob[:])

                pending = [emit_score(0, t) for t in range(NT)]
                for step in range(SC * H):
                    c, h = divmod(step, H)
                    cur = pending
                    nxt = []
                    ps_ones = ps.tile([1, 512], F32, tag="ps")
                    ps_av = ps.tile([P, 512], F32, tag="ps")
                    for t in range(NT):
                        if step + 1 < SC * H:
                            nxt.append(emit_score(step + 1, t))
                        nc.tensor.matmul(ps_ones[:], ones_t[:], cur[t][:],
                                         start=(t == 0), stop=(t == NT - 1))
                        nc.tensor.matmul(ps_av[:], v_tiles[t][:], cur[t][:],
                                         start=(t == 0), stop=(t == NT - 1))
                    rc = smp.tile([1, 512], F32, tag="rc", bufs=2)
                    nc.vector.reciprocal(rc[:], ps_ones[:])
                    rc_r = smp.tile([1, 512], F32R, tag="rc_r", bufs=2)
                    nc.vector.tensor_copy(rc_r[:], rc[:])
                    ps_rcb = ps.tile([P, 512], F32, tag="ps")
                    nc.tensor.matmul(ps_rcb[:], ones_col[:], rc_r[:],
                                     start=True, stop=True)
                    rcb = smp.tile([P, 512], F32, tag="rcb", bufs=2)
                    nc.scalar.activation(rcb[:], ps_rcb[:], COPYF)
                    nc.vector.tensor_mul(
                        attnT[h][:, c * 512:(c + 1) * 512], ps_av[:], rcb[:])
                    pending = nxt
                    if h == H - 1:
                        emit_wo(c)

            if loop_trips is None:
                body()
            else:
                with tc.For_i(0, loop_trips, 1) as iv:
                    body(iv)

    nc.compile()
    return nc


_CACHE = {}


def _get_nc():
    if "nc" not in _CACHE:
        _CACHE["nc"] = build()
    return _CACHE["nc"]


def make_in_maps(query, key_in, value, Wq, bq, Wk, bk, Wv, bv, Wo, bo):
    f32 = np.float32
    in_maps = []
    xT = {}
    for b in range(B):
        xT[b] = (
            np.ascontiguousarray(np.asarray(query[b], f32).T),
            np.ascontiguousarray(np.asarray(key_in[b], f32).T),
            np.ascontiguousarray(np.asarray(value[b], f32).T),
        )
    Wq, Wk, Wv, Wo = (np.asarray(a, f32) for a in (Wq, Wk, Wv, Wo))
    bq, bk, bv = (np.asarray(a, f32) for a in (bq, bk, bv))
    for core in range(N_CORES):
        b, g = divmod(core, G)
        xq, xk, xv = xT[b]
        in_maps.append({
            "xq": xq, "xk": xk, "xv": xv,
            "wq": np.ascontiguousarray(Wq[:, g * MQ:(g + 1) * MQ]),
            "wk": np.ascontiguousarray(Wk[:, g * HD:(g + 1) * HD]),
            "wv": np.ascontiguousarray(Wv[:, g * HD:(g + 1) * HD]),
            "wo": np.ascontiguousarray(Wo[g * MQ:(g + 1) * MQ, :]),
            "bq": np.ascontiguousarray(bq[g * MQ:(g + 1) * MQ].reshape(MQ, 1)),
            "bk": np.ascontiguousarray(bk[g * HD:(g + 1) * HD].reshape(HD, 1)),
            "bv": np.ascontiguousarray(bv[g * HD:(g + 1) * HD].reshape(HD, 1)),
        })
    return in_maps


def assemble(results, bo):
    bo = np.asarray(bo, np.float32)
    out = np.empty((B, S, E), np.float32)
    for b in range(B):
        acc = results[b * G]["out"].astype(np.float32)
        for g in range(1, G):
            acc = acc + results[b * G + g]["out"]
        out[b] = acc + bo[None, :]
    return out


def kernel(query, key_in, value, Wq, bq, Wk, bk, Wv, bv, Wo, bo):
    from concourse.bass_utils import run_bass_kernel_spmd
    nc = _get_nc()
    in_maps = make_in_maps(query, key_in, value, Wq, bq, Wk, bk, Wv, bv, Wo, bo)
    res = run_bass_kernel_spmd(nc, in_maps, core_ids=list(range(N_CORES)))
    return assemble(res.results, bo)
